# revision 7
# baseline (speedup 1.0000x reference)
"""HNHN hypergraph message passing on 8 Trainium2 NeuronCores.

Math (biases folded through the linear maps, which commute with segment_sum):
  Wc = W1 @ Wve ; bc = b1 @ Wve + bve
  feat_e[e]  = (sum_{k:e_k=e} w_in[k] * vfeat[n_k]) @ Wc + (sum w_in[k]) * bc
  Wh_e       = feat_e @ Wev + bev
  feat_v_out[n] = sum_{k:n_k=n} w_con[k] * Wh_e[e_k]

Device strategy (graph/data parallel, per sharding hint):
  Phase B: incidence list sorted by edge, sharded into contiguous edge
  ranges per core. Each core indirect-DMA-gathers vfeat rows per message
  tile [128 msgs x 128], builds a weighted selection matrix M_w[k,j] =
  w_k * (el_k == j) on DVE, and accumulates aggT[c, e] in PSUM via
  matmul(lhsT=msgs, rhs=M_w). Projection through Wc/bc, then Wh_e^T,
  PE-transposed to rows, AllGather -> full Wh_e row table (bf16).
  Phase D: incidence list sorted by node; gather Wh_e rows; matmul
  (lhsT=M_w, rhs=msgs) accumulates feat_v_out rows per node window.
"""
import sys
sys.path.insert(0, "/opt/trn_rl_repo")
import numpy as np
import ml_dtypes

import concourse.bass as bass
import concourse.mybir as mybir
import concourse.tile as tile
from concourse.masks import make_identity

P = 128
NCORES = 8

_DMA_OPCODES = ("DMACopy", "DMA", "DMAGatherAnt", "DMAScatterAddAnt",
                "DmaTransposeAnt", "TensorLoad", "TensorSave", "Drain")


def _split_dma_waits(nc):
    """walrus DMA pseudo-instructions accept a single sync-wait; hoist
    extras onto standalone EventSemaphore ops on the same engine stream."""
    n = 0
    for f in nc.m.functions:
        for blk in f.blocks:
            if not any(
                i.sync_info is not None and i.sync_info.on_wait
                and len(i.sync_info.on_wait) > 1 and i.opcode != "EventSemaphore"
                for i in blk.instructions
            ):
                continue
            newinsts = []
            for inst in blk.instructions:
                si = inst.sync_info
                if (si is not None and si.on_wait and len(si.on_wait) > 1
                        and inst.opcode != "EventSemaphore"):
                    for w in si.on_wait:
                        n += 1
                        newinsts.append(mybir.InstEventSemaphore(
                            name=f"waitfix_{n}_{inst.name}",
                            opcode="EventSemaphore", engine=inst.engine,
                            ins=[], outs=[],
                            sync_info=mybir.SyncInfo(on_wait=[w], on_update=[])))
                    inst.sync_info = mybir.SyncInfo(on_wait=[], on_update=si.on_update)
                newinsts.append(inst)
            blk.instructions = newinsts
    return n


def _plan(seg_ids, gather_rows, w, n_seg_pc, n_win_pc, bf16_w=False):
    """Pack messages (already sorted by seg_ids) into fixed-shape per-core
    planes: gidx/el/w of shape [NCORES, P, NT]. Window = 128 consecutive
    segments; every window gets the same tile count Twin (max over all)."""
    nnz = seg_ids.shape[0]
    core = seg_ids // n_seg_pc
    local = seg_ids - core * n_seg_pc
    win = local // P
    el = (local % P).astype(np.float32)
    g = core * n_win_pc + win
    counts = np.bincount(g, minlength=NCORES * n_win_pc)
    Twin = max(1, int(-(-counts.max() // P)))
    starts = np.concatenate([[0], np.cumsum(counts)[:-1]])
    off = np.arange(nnz) - starts[g]
    slot = win * (Twin * P) + off
    t_i = (slot // P).astype(np.int64)
    part = (slot % P).astype(np.int64)
    NT = n_win_pc * Twin
    gidx = np.zeros((NCORES, P, NT), np.int32)
    elp = np.zeros((NCORES, P, NT), np.float32)
    wp = np.zeros((NCORES, P, NT), np.float32)
    gidx[core, part, t_i] = gather_rows
    elp[core, part, t_i] = el
    wp[core, part, t_i] = w
    return gidx, elp, wp, Twin, NT


def _build(NV, NE, NT_B, NT_D, E_pad, N_pad, WB, TB, WD, TD):
    f32, bf16, i32 = mybir.dt.float32, mybir.dt.bfloat16, mybir.dt.int32
    nc = bass.Bass()
    vfeat = nc.declare_dram_parameter("vfeat", [NV, P], f32, isOutput=False)
    gidxB = nc.declare_dram_parameter("gidxB", [P, NT_B], i32, isOutput=False)
    elB = nc.declare_dram_parameter("elB", [P, NT_B], f32, isOutput=False)
    wB = nc.declare_dram_parameter("wB", [P, NT_B], f32, isOutput=False)
    sumw = nc.declare_dram_parameter("sumw", [1, E_pad], bf16, isOutput=False)
    gidxD = nc.declare_dram_parameter("gidxD", [P, NT_D], i32, isOutput=False)
    elD = nc.declare_dram_parameter("elD", [P, NT_D], f32, isOutput=False)
    wD = nc.declare_dram_parameter("wD", [P, NT_D], f32, isOutput=False)
    Wc = nc.declare_dram_parameter("Wc", [P, P], bf16, isOutput=False)
    bc = nc.declare_dram_parameter("bc", [1, P], bf16, isOutput=False)
    Wev = nc.declare_dram_parameter("Wev", [P, P], bf16, isOutput=False)
    bev = nc.declare_dram_parameter("bev", [1, P], bf16, isOutput=False)
    feT = nc.declare_dram_parameter("feT", [P, E_pad], f32, isOutput=True)
    fvout = nc.declare_dram_parameter("fvout", [N_pad, P], f32, isOutput=True)
    whe_sh = nc.dram_tensor("whe_sh", [E_pad, P], bf16)
    whe_full = nc.dram_tensor("whe_full", [NCORES * E_pad, P], bf16, addr_space="Shared")
    fvv = fvout[:].rearrange("(w n) d -> w n d", n=P)
    whv = whe_sh[:].rearrange("(b r) d -> b r d", r=P)

    with tile.TileContext(nc) as tc:
        with (
            tc.tile_pool(name="meta", bufs=1) as meta,
            tc.tile_pool(name="cst", bufs=1) as cst,
            tc.tile_pool(name="agg", bufs=1) as aggp,
            tc.tile_pool(name="g32", bufs=12) as g32p,
            tc.tile_pool(name="g16", bufs=8) as g16p,
            tc.tile_pool(name="mw", bufs=8) as mwp,
            tc.tile_pool(name="st", bufs=4) as stp,
            tc.tile_pool(name="psA", bufs=2, space="PSUM") as psA,
            tc.tile_pool(name="psP", bufs=1, space="PSUM") as psP,
            tc.tile_pool(name="psT", bufs=2, space="PSUM") as psT,
            tc.tile_pool(name="psD", bufs=2, space="PSUM") as psD,
        ):
            # --- constants & metadata (resident) ---
            ident = cst.tile([P, P], f32)
            make_identity(nc, ident[:])
            iota_i = cst.tile([P, P], i32)
            nc.gpsimd.iota(iota_i[:], pattern=[[1, P]], channel_multiplier=0)
            iota_f = cst.tile([P, P], f32)
            nc.vector.tensor_copy(iota_f[:], iota_i[:])
            ones = cst.tile([1, 512], bf16)
            nc.vector.memset(ones[:], 1.0)
            Wc_sb = cst.tile([P, P], bf16)
            nc.sync.dma_start(Wc_sb[:], Wc[:])
            bc_sb = cst.tile([1, P], bf16)
            nc.sync.dma_start(bc_sb[:], bc[:])
            Wev_sb = cst.tile([P, P], bf16)
            nc.sync.dma_start(Wev_sb[:], Wev[:])
            bev_sb = cst.tile([1, P], bf16)
            nc.sync.dma_start(bev_sb[:], bev[:])
            sumw_sb = cst.tile([1, E_pad], bf16)
            nc.sync.dma_start(sumw_sb[:], sumw[:])
            gB_sb = meta.tile([P, NT_B], i32)
            nc.sync.dma_start(gB_sb[:], gidxB[:])
            eB_sb = meta.tile([P, NT_B], f32)
            nc.sync.dma_start(eB_sb[:], elB[:])
            wB_sb = meta.tile([P, NT_B], f32)
            nc.sync.dma_start(wB_sb[:], wB[:])
            gD_sb = meta.tile([P, NT_D], i32)
            nc.sync.dma_start(gD_sb[:], gidxD[:])
            eD_sb = meta.tile([P, NT_D], f32)
            nc.sync.dma_start(eD_sb[:], elD[:])
            wD_sb = meta.tile([P, NT_D], f32)
            nc.sync.dma_start(wD_sb[:], wD[:])
            aggT = aggp.tile([P, E_pad], bf16)

            # --- Phase B: aggregate w*vfeat rows by edge window ---
            for w in range(WB):
                ps = psA.tile([P, P], f32)
                for t in range(TB):
                    ti = w * TB + t
                    g16 = g16p.tile([P, P], bf16)
                    nc.gpsimd.indirect_dma_start(
                        out=g16[:], out_offset=None, in_=vfeat[:].bitcast_dtype_unsafe(bf16) if False else vfeat[:],
                        in_offset=bass.IndirectOffsetOnAxis(
                            ap=gB_sb[:, ti:ti + 1], axis=0))
                    mw = mwp.tile([P, P], bf16)
                    nc.vector.tensor_scalar(
                        mw[:], iota_f[:], eB_sb[:, ti:ti + 1], wB_sb[:, ti:ti + 1],
                        mybir.AluOpType.is_equal, mybir.AluOpType.mult)
                    nc.tensor.matmul(ps[:], lhsT=g16[:], rhs=mw[:],
                                     start=(t == 0), stop=(t == TB - 1))
                nc.vector.tensor_copy(aggT[:, w * P:(w + 1) * P], ps[:])

            # --- Phase B projection: feat_e^T, Wh_e^T -> rows -> AllGather ---
            for ch in range(E_pad // 512):
                cs = slice(ch * 512, (ch + 1) * 512)
                pfe = psP.tile([P, 512], f32)
                nc.tensor.matmul(pfe[:], lhsT=Wc_sb[:], rhs=aggT[:, cs],
                                 start=True, stop=False)
                nc.tensor.matmul(pfe[:], lhsT=bc_sb[:], rhs=sumw_sb[:, cs],
                                 start=False, stop=True)
                fe32 = stp.tile([P, 512], f32)
                nc.vector.tensor_copy(fe32[:], pfe[:])
                nc.sync.dma_start(feT[:, cs], fe32[:])
                fe16 = stp.tile([P, 512], bf16)
                nc.scalar.copy(fe16[:], pfe[:])
                pwh = psP.tile([P, 512], f32)
                nc.tensor.matmul(pwh[:], lhsT=Wev_sb[:], rhs=fe16[:],
                                 start=True, stop=False)
                nc.tensor.matmul(pwh[:], lhsT=bev_sb[:], rhs=ones[:],
                                 start=False, stop=True)
                wh32 = stp.tile([P, 512], f32)
                nc.vector.tensor_copy(wh32[:], pwh[:])
                for j in range(4):
                    ptr = psT.tile([P, P], f32)
                    nc.tensor.transpose(ptr[:], in_=wh32[:, j * P:(j + 1) * P],
                                        identity=ident[:])
                    tr16 = stp.tile([P, P], bf16)
                    nc.vector.tensor_copy(tr16[:], ptr[:])
                    nc.sync.dma_start(whv[ch * 4 + j], tr16[:])

            nc.gpsimd.collective_compute(
                "AllGather", mybir.AluOpType.bypass,
                replica_groups=[list(range(NCORES))],
                ins=[whe_sh[:]], outs=[whe_full[:]])

            # --- Phase D: gather Wh_e rows, aggregate into node windows ---
            for w in range(WD):
                ps = psD.tile([P, P], f32)
                for t in range(TD):
                    ti = w * TD + t
                    g16 = g16p.tile([P, P], bf16)
                    nc.gpsimd.indirect_dma_start(
                        out=g16[:], out_offset=None, in_=whe_full[:],
                        in_offset=bass.IndirectOffsetOnAxis(
                            ap=gD_sb[:, ti:ti + 1], axis=0))
                    mw = mwp.tile([P, P], bf16)
                    nc.vector.tensor_scalar(
                        mw[:], iota_f[:], eD_sb[:, ti:ti + 1], wD_sb[:, ti:ti + 1],
                        mybir.AluOpType.is_equal, mybir.AluOpType.mult)
                    nc.tensor.matmul(ps[:], lhsT=mw[:], rhs=g16[:],
                                     start=(t == 0), stop=(t == TD - 1))
                o32 = stp.tile([P, P], f32)
                nc.vector.tensor_copy(o32[:], ps[:])
                nc.sync.dma_start(fvv[w], o32[:])

    nc.finalize()
    _split_dma_waits(nc)
    return nc


class _Runner:
    def __init__(self, nc, n_cores=NCORES):
        import jax
        from jax.sharding import Mesh, PartitionSpec
        from jax.experimental.shard_map import shard_map
        from concourse.bass2jax import (_bass_exec_p, install_neuronx_cc_hook,
                                        partition_id_tensor)
        install_neuronx_cc_hook()
        self.jax, self.n_cores = jax, n_cores
        pname = nc.partition_id_tensor.name if nc.partition_id_tensor else None
        in_names, out_names, out_avals, zero_outs = [], [], [], []
        for alloc in nc.m.functions[0].allocations:
            if not isinstance(alloc, mybir.MemoryLocationSet):
                continue
            name = alloc.memorylocations[0].name
            if alloc.kind == "ExternalInput":
                if name != pname:
                    in_names.append(name)
            elif alloc.kind == "ExternalOutput":
                shape = list(alloc.tensor_shape)
                np_dt = mybir.dt.np(alloc.dtype)
                out_avals.append(jax.core.ShapedArray(shape, np_dt))
                out_names.append(name)
                zero_outs.append(np.zeros(shape, np_dt))
        self.in_names, self.out_names = in_names, out_names
        self.out_avals, self.zero_outs = out_avals, zero_outs
        all_in = in_names + out_names + ([pname] if pname else [])

        def _body(*args):
            operands = list(args)
            if pname is not None:
                operands.append(partition_id_tensor())
            return tuple(_bass_exec_p.bind(
                *operands, out_avals=tuple(out_avals), in_names=tuple(all_in),
                out_names=tuple(out_names), lowering_input_output_aliases=(),
                sim_require_finite=False, sim_require_nnan=False, nc=nc))

        self._body = _body
        devices = jax.devices()[:n_cores]
        self.mesh = Mesh(np.asarray(devices), ("core",))
        nin = len(in_names) + len(out_names)
        self.fn = jax.jit(shard_map(
            _body, mesh=self.mesh, in_specs=(PartitionSpec("core"),) * nin,
            out_specs=(PartitionSpec("core"),) * len(out_names), check_rep=False))

    def put_inputs(self, in_maps):
        from jax.sharding import PartitionSpec
        concat = [np.concatenate([np.asarray(in_maps[c][n])
                                  for c in range(self.n_cores)], axis=0)
                  for n in self.in_names]
        concat += [np.zeros((self.n_cores * z.shape[0], *z.shape[1:]), z.dtype)
                   for z in self.zero_outs]
        sh = self.jax.sharding.NamedSharding(self.mesh, PartitionSpec("core"))
        return [self.jax.device_put(a, sh) for a in concat]

    def run(self, dev):
        outs = self.fn(*dev)
        self.jax.block_until_ready(outs)
        return outs

    def results(self, outs):
        return [
            {n: np.asarray(outs[i]).reshape(self.n_cores, *self.out_avals[i].shape)[c]
             for i, n in enumerate(self.out_names)}
            for c in range(self.n_cores)
        ]


_CACHE = {}
_LAST_DEV = None


def kernel(vfeat, efeat, v_reg_weight, v_reg_sum, e_reg_weight, e_reg_sum,
           node_idx, edge_idx, W1, b1, Wve, bve, Wev, bev,
           first_layer, last_layer):
    vfeat = np.asarray(vfeat, np.float32)
    NV, D = vfeat.shape
    NE = int(np.asarray(e_reg_weight).shape[0])
    node_idx = np.asarray(node_idx).astype(np.int64).ravel()
    edge_idx = np.asarray(edge_idx).astype(np.int64).ravel()

    # Host: weight fusion + per-message scalar weights (index metadata prep)
    W1f = np.asarray(W1, np.float32)
    Wvef = np.asarray(Wve, np.float32)
    Wc = (W1f @ Wvef)
    bcv = (np.asarray(b1, np.float32) @ Wvef + np.asarray(bve, np.float32))
    vrw = np.asarray(v_reg_weight, np.float32).ravel()
    vrs = np.asarray(v_reg_sum, np.float32).ravel()
    erw = np.asarray(e_reg_weight, np.float32).ravel()
    ers = np.asarray(e_reg_sum, np.float32).ravel()
    w_in = vrw[node_idx] / ers[edge_idx]
    w_con = erw[edge_idx] / vrs[node_idx]
    sumw = np.bincount(edge_idx, weights=w_in, minlength=NE).astype(np.float32)

    E_pc, N_pc = NE // NCORES, NV // NCORES
    WB = -(-E_pc // P)
    E_pad = WB * P
    WD = -(-N_pc // P)
    N_pad = WD * P

    oB = np.argsort(edge_idx, kind="stable")
    gidxB, elB, wB, TB, NT_B = _plan(
        edge_idx[oB], node_idx[oB].astype(np.int32), w_in[oB], E_pc, WB)
    oD = np.argsort(node_idx, kind="stable")
    ce = (edge_idx[oD] // E_pc).astype(np.int64)
    growD = (ce * E_pad + edge_idx[oD] - ce * E_pc).astype(np.int32)
    gidxD, elD, wD, TD, NT_D = _plan(
        node_idx[oD], growD, w_con[oD], N_pc, WD)

    key = (NV, NE, NT_B, NT_D, E_pad, N_pad, WB, TB, WD, TD)
    if key not in _CACHE:
        nc = _build(*key)
        _CACHE[key] = _Runner(nc)
    r = _CACHE[key]

    bf = ml_dtypes.bfloat16
    sumw_pad = np.zeros((NCORES, 1, E_pad), np.float32)
    sumw_pad[:, 0, :E_pc] = sumw.reshape(NCORES, E_pc)
    in_maps = []
    for c in range(NCORES):
        in_maps.append({
            "vfeat": vfeat,
            "gidxB": gidxB[c], "elB": elB[c], "wB": wB[c],
            "sumw": sumw_pad[c].astype(bf),
            "gidxD": gidxD[c], "elD": elD[c], "wD": wD[c],
            "Wc": Wc.astype(bf), "bc": bcv.reshape(1, P).astype(bf),
            "Wev": np.asarray(Wev, np.float32).astype(bf),
            "bev": np.asarray(bev, np.float32).reshape(1, P).astype(bf),
        })
    dev = r.put_inputs(in_maps)
    global _LAST_DEV
    _LAST_DEV = dev
    outs = r.run(dev)
    res = r.results(outs)

    feat_e = np.concatenate([res[c]["feT"].T[:E_pc] for c in range(NCORES)], 0)
    feat_v = np.concatenate([res[c]["fvout"][:N_pc] for c in range(NCORES)], 0)
    return feat_v.astype(np.float32), feat_e.astype(np.float32)


# revision 8
# speedup vs baseline: 1.6422x; 1.6422x over previous
"""HNHN hypergraph message passing on 8 Trainium2 NeuronCores.

Math (biases folded through the linear maps, which commute with segment_sum):
  Wc = W1 @ Wve ; bc = b1 @ Wve + bve
  feat_e[e]  = (sum_{k:e_k=e} w_in[k] * vfeat[n_k]) @ Wc + (sum w_in[k]) * bc
  Wh_e       = feat_e @ Wev + bev
  feat_v_out[n] = sum_{k:n_k=n} w_con[k] * Wh_e[e_k]

Device strategy (graph/data parallel, per sharding hint):
  Phase B: incidence list sorted by edge, sharded into contiguous edge
  ranges per core. Each core indirect-DMA-gathers vfeat rows per message
  tile [128 msgs x 128], builds a weighted selection matrix M_w[k,j] =
  w_k * (el_k == j) on DVE, and accumulates aggT[c, e] in PSUM via
  matmul(lhsT=msgs, rhs=M_w). Projection through Wc/bc, then Wh_e^T,
  PE-transposed to rows, AllGather -> full Wh_e row table (bf16).
  Phase D: incidence list sorted by node; gather Wh_e rows; matmul
  (lhsT=M_w, rhs=msgs) accumulates feat_v_out rows per node window.
"""
import sys
sys.path.insert(0, "/opt/trn_rl_repo")
import numpy as np
import ml_dtypes

import concourse.bass as bass
import concourse.mybir as mybir
import concourse.tile as tile
from concourse.masks import make_identity

P = 128
NCORES = 8

_DMA_OPCODES = ("DMACopy", "DMA", "DMAGatherAnt", "DMAScatterAddAnt",
                "DmaTransposeAnt", "TensorLoad", "TensorSave", "Drain")


def _split_dma_waits(nc):
    """walrus DMA pseudo-instructions accept a single sync-wait; hoist
    extras onto standalone EventSemaphore ops on the same engine stream."""
    n = 0
    for f in nc.m.functions:
        for blk in f.blocks:
            if not any(
                i.sync_info is not None and i.sync_info.on_wait
                and len(i.sync_info.on_wait) > 1 and i.opcode != "EventSemaphore"
                for i in blk.instructions
            ):
                continue
            newinsts = []
            for inst in blk.instructions:
                si = inst.sync_info
                if (si is not None and si.on_wait and len(si.on_wait) > 1
                        and inst.opcode != "EventSemaphore"):
                    for w in si.on_wait:
                        n += 1
                        newinsts.append(mybir.InstEventSemaphore(
                            name=f"waitfix_{n}_{inst.name}",
                            opcode="EventSemaphore", engine=inst.engine,
                            ins=[], outs=[],
                            sync_info=mybir.SyncInfo(on_wait=[w], on_update=[])))
                    inst.sync_info = mybir.SyncInfo(on_wait=[], on_update=si.on_update)
                newinsts.append(inst)
            blk.instructions = newinsts
    return n


def _plan(seg_ids, gather_rows, w, n_seg_pc, n_win_pc, bf16_w=False):
    """Pack messages (already sorted by seg_ids) into fixed-shape per-core
    planes: gidx/el/w of shape [NCORES, P, NT]. Window = 128 consecutive
    segments; every window gets the same tile count Twin (max over all)."""
    nnz = seg_ids.shape[0]
    core = seg_ids // n_seg_pc
    local = seg_ids - core * n_seg_pc
    win = local // P
    el = (local % P).astype(np.float32)
    g = core * n_win_pc + win
    counts = np.bincount(g, minlength=NCORES * n_win_pc)
    Twin = max(1, int(-(-counts.max() // P)))
    starts = np.concatenate([[0], np.cumsum(counts)[:-1]])
    off = np.arange(nnz) - starts[g]
    slot = win * (Twin * P) + off
    t_i = (slot // P).astype(np.int64)
    part = (slot % P).astype(np.int64)
    NT = n_win_pc * Twin
    gidx = np.zeros((NCORES, P, NT), np.int32)
    elp = np.zeros((NCORES, P, NT), np.float32)
    wp = np.zeros((NCORES, P, NT), np.float32)
    gidx[core, part, t_i] = gather_rows
    elp[core, part, t_i] = el
    wp[core, part, t_i] = w
    return gidx, elp, wp, Twin, NT


def _build(NV, NE, NT_B, NT_D, E_pad, N_pad, WB, TB, WD, TD):
    f32, bf16, i32 = mybir.dt.float32, mybir.dt.bfloat16, mybir.dt.int32
    nc = bass.Bass()
    vfeat = nc.declare_dram_parameter("vfeat", [NV, P], f32, isOutput=False)
    gidxB = nc.declare_dram_parameter("gidxB", [P, NT_B], i32, isOutput=False)
    elB = nc.declare_dram_parameter("elB", [P, NT_B], f32, isOutput=False)
    wB = nc.declare_dram_parameter("wB", [P, NT_B], f32, isOutput=False)
    sumw = nc.declare_dram_parameter("sumw", [1, E_pad], bf16, isOutput=False)
    gidxD = nc.declare_dram_parameter("gidxD", [P, NT_D], i32, isOutput=False)
    elD = nc.declare_dram_parameter("elD", [P, NT_D], f32, isOutput=False)
    wD = nc.declare_dram_parameter("wD", [P, NT_D], f32, isOutput=False)
    Wc = nc.declare_dram_parameter("Wc", [P, P], bf16, isOutput=False)
    bc = nc.declare_dram_parameter("bc", [1, P], bf16, isOutput=False)
    Wev = nc.declare_dram_parameter("Wev", [P, P], bf16, isOutput=False)
    bev = nc.declare_dram_parameter("bev", [1, P], bf16, isOutput=False)
    feT = nc.declare_dram_parameter("feT", [P, E_pad], f32, isOutput=True)
    fvout = nc.declare_dram_parameter("fvout", [N_pad, P], f32, isOutput=True)
    whe_sh = nc.dram_tensor("whe_sh", [E_pad, P], bf16)
    whe_full = nc.dram_tensor("whe_full", [NCORES * E_pad, P], bf16, addr_space="Shared")
    fvv = fvout[:].rearrange("(w n) d -> w n d", n=P)
    whv = whe_sh[:].rearrange("(b r) d -> b r d", r=P)

    with tile.TileContext(nc) as tc:
        with (
            tc.tile_pool(name="meta", bufs=1) as meta,
            tc.tile_pool(name="cst", bufs=1) as cst,
            tc.tile_pool(name="agg", bufs=1) as aggp,
            tc.tile_pool(name="g32", bufs=12) as g32p,
            tc.tile_pool(name="g16", bufs=8) as g16p,
            tc.tile_pool(name="mw", bufs=8) as mwp,
            tc.tile_pool(name="st", bufs=4) as stp,
            tc.tile_pool(name="psA", bufs=2, space="PSUM") as psA,
            tc.tile_pool(name="psP", bufs=1, space="PSUM") as psP,
            tc.tile_pool(name="psT", bufs=2, space="PSUM") as psT,
            tc.tile_pool(name="psD", bufs=2, space="PSUM") as psD,
        ):
            # --- constants & metadata (resident) ---
            ident = cst.tile([P, P], f32)
            make_identity(nc, ident[:])
            iota_i = cst.tile([P, P], i32)
            nc.gpsimd.iota(iota_i[:], pattern=[[1, P]], channel_multiplier=0)
            iota_f = cst.tile([P, P], f32)
            nc.vector.tensor_copy(iota_f[:], iota_i[:])
            ones = cst.tile([1, 512], bf16)
            nc.vector.memset(ones[:], 1.0)
            Wc_sb = cst.tile([P, P], bf16)
            nc.sync.dma_start(Wc_sb[:], Wc[:])
            bc_sb = cst.tile([1, P], bf16)
            nc.sync.dma_start(bc_sb[:], bc[:])
            Wev_sb = cst.tile([P, P], bf16)
            nc.sync.dma_start(Wev_sb[:], Wev[:])
            bev_sb = cst.tile([1, P], bf16)
            nc.sync.dma_start(bev_sb[:], bev[:])
            sumw_sb = cst.tile([1, E_pad], bf16)
            nc.sync.dma_start(sumw_sb[:], sumw[:])
            gB_sb = meta.tile([P, NT_B], i32)
            nc.sync.dma_start(gB_sb[:], gidxB[:])
            eB_sb = meta.tile([P, NT_B], f32)
            nc.sync.dma_start(eB_sb[:], elB[:])
            wB_sb = meta.tile([P, NT_B], f32)
            nc.sync.dma_start(wB_sb[:], wB[:])
            gD_sb = meta.tile([P, NT_D], i32)
            nc.sync.dma_start(gD_sb[:], gidxD[:])
            eD_sb = meta.tile([P, NT_D], f32)
            nc.sync.dma_start(eD_sb[:], elD[:])
            wD_sb = meta.tile([P, NT_D], f32)
            nc.sync.dma_start(wD_sb[:], wD[:])
            aggT = aggp.tile([P, E_pad], bf16)

            # --- Phase B: aggregate w*vfeat rows by edge window ---
            for w in range(WB):
                ps = psA.tile([P, P], f32)
                for t in range(TB):
                    ti = w * TB + t
                    g32 = g32p.tile([P, P], f32)
                    nc.gpsimd.indirect_dma_start(
                        out=g32[:], out_offset=None, in_=vfeat[:],
                        in_offset=bass.IndirectOffsetOnAxis(
                            ap=gB_sb[:, ti:ti + 1], axis=0))
                    g16 = g16p.tile([P, P], bf16)
                    nc.scalar.copy(g16[:], g32[:])
                    mw = mwp.tile([P, P], bf16)
                    nc.vector.tensor_scalar(
                        mw[:], iota_f[:], eB_sb[:, ti:ti + 1], wB_sb[:, ti:ti + 1],
                        mybir.AluOpType.is_equal, mybir.AluOpType.mult)
                    nc.tensor.matmul(ps[:], lhsT=g16[:], rhs=mw[:],
                                     start=(t == 0), stop=(t == TB - 1))
                nc.vector.tensor_copy(aggT[:, w * P:(w + 1) * P], ps[:])

            # --- Phase B projection: feat_e^T, Wh_e^T -> rows -> AllGather ---
            for ch in range(E_pad // 512):
                cs = slice(ch * 512, (ch + 1) * 512)
                pfe = psP.tile([P, 512], f32)
                nc.tensor.matmul(pfe[:], lhsT=Wc_sb[:], rhs=aggT[:, cs],
                                 start=True, stop=False)
                nc.tensor.matmul(pfe[:], lhsT=bc_sb[:], rhs=sumw_sb[:, cs],
                                 start=False, stop=True)
                fe32 = stp.tile([P, 512], f32)
                nc.vector.tensor_copy(fe32[:], pfe[:])
                nc.sync.dma_start(feT[:, cs], fe32[:])
                fe16 = stp.tile([P, 512], bf16)
                nc.scalar.copy(fe16[:], pfe[:])
                pwh = psP.tile([P, 512], f32)
                nc.tensor.matmul(pwh[:], lhsT=Wev_sb[:], rhs=fe16[:],
                                 start=True, stop=False)
                nc.tensor.matmul(pwh[:], lhsT=bev_sb[:], rhs=ones[:],
                                 start=False, stop=True)
                wh32 = stp.tile([P, 512], f32)
                nc.vector.tensor_copy(wh32[:], pwh[:])
                for j in range(4):
                    ptr = psT.tile([P, P], f32)
                    nc.tensor.transpose(ptr[:], in_=wh32[:, j * P:(j + 1) * P],
                                        identity=ident[:])
                    tr16 = stp.tile([P, P], bf16)
                    nc.vector.tensor_copy(tr16[:], ptr[:])
                    nc.sync.dma_start(whv[ch * 4 + j], tr16[:])

            nc.gpsimd.collective_compute(
                "AllGather", mybir.AluOpType.bypass,
                replica_groups=[list(range(NCORES))],
                ins=[whe_sh[:]], outs=[whe_full[:]])

            # --- Phase D: gather Wh_e rows, aggregate into node windows ---
            for w in range(WD):
                ps = psD.tile([P, P], f32)
                for t in range(TD):
                    ti = w * TD + t
                    g16 = g16p.tile([P, P], bf16)
                    nc.gpsimd.indirect_dma_start(
                        out=g16[:], out_offset=None, in_=whe_full[:],
                        in_offset=bass.IndirectOffsetOnAxis(
                            ap=gD_sb[:, ti:ti + 1], axis=0))
                    mw = mwp.tile([P, P], bf16)
                    nc.vector.tensor_scalar(
                        mw[:], iota_f[:], eD_sb[:, ti:ti + 1], wD_sb[:, ti:ti + 1],
                        mybir.AluOpType.is_equal, mybir.AluOpType.mult)
                    nc.tensor.matmul(ps[:], lhsT=mw[:], rhs=g16[:],
                                     start=(t == 0), stop=(t == TD - 1))
                o32 = stp.tile([P, P], f32)
                nc.vector.tensor_copy(o32[:], ps[:])
                nc.sync.dma_start(fvv[w], o32[:])

    nc.finalize()
    _split_dma_waits(nc)
    return nc


class _Runner:
    def __init__(self, nc, n_cores=NCORES):
        import jax
        from jax.sharding import Mesh, PartitionSpec
        from jax.experimental.shard_map import shard_map
        from concourse.bass2jax import (_bass_exec_p, install_neuronx_cc_hook,
                                        partition_id_tensor)
        install_neuronx_cc_hook()
        self.jax, self.n_cores = jax, n_cores
        pname = nc.partition_id_tensor.name if nc.partition_id_tensor else None
        in_names, out_names, out_avals, zero_outs = [], [], [], []
        for alloc in nc.m.functions[0].allocations:
            if not isinstance(alloc, mybir.MemoryLocationSet):
                continue
            name = alloc.memorylocations[0].name
            if alloc.kind == "ExternalInput":
                if name != pname:
                    in_names.append(name)
            elif alloc.kind == "ExternalOutput":
                shape = list(alloc.tensor_shape)
                np_dt = mybir.dt.np(alloc.dtype)
                out_avals.append(jax.core.ShapedArray(shape, np_dt))
                out_names.append(name)
                zero_outs.append(np.zeros(shape, np_dt))
        self.in_names, self.out_names = in_names, out_names
        self.out_avals, self.zero_outs = out_avals, zero_outs
        all_in = in_names + out_names + ([pname] if pname else [])

        def _body(*args):
            operands = list(args)
            if pname is not None:
                operands.append(partition_id_tensor())
            return tuple(_bass_exec_p.bind(
                *operands, out_avals=tuple(out_avals), in_names=tuple(all_in),
                out_names=tuple(out_names), lowering_input_output_aliases=(),
                sim_require_finite=False, sim_require_nnan=False, nc=nc))

        self._body = _body
        devices = jax.devices()[:n_cores]
        self.mesh = Mesh(np.asarray(devices), ("core",))
        nin = len(in_names) + len(out_names)
        self.fn = jax.jit(shard_map(
            _body, mesh=self.mesh, in_specs=(PartitionSpec("core"),) * nin,
            out_specs=(PartitionSpec("core"),) * len(out_names), check_rep=False))

    def put_inputs(self, in_maps):
        from jax.sharding import PartitionSpec
        concat = [np.concatenate([np.asarray(in_maps[c][n])
                                  for c in range(self.n_cores)], axis=0)
                  for n in self.in_names]
        concat += [np.zeros((self.n_cores * z.shape[0], *z.shape[1:]), z.dtype)
                   for z in self.zero_outs]
        sh = self.jax.sharding.NamedSharding(self.mesh, PartitionSpec("core"))
        return [self.jax.device_put(a, sh) for a in concat]

    def run(self, dev):
        outs = self.fn(*dev)
        self.jax.block_until_ready(outs)
        return outs

    def results(self, outs):
        return [
            {n: np.asarray(outs[i]).reshape(self.n_cores, *self.out_avals[i].shape)[c]
             for i, n in enumerate(self.out_names)}
            for c in range(self.n_cores)
        ]


_CACHE = {}
_LAST_DEV = None


def kernel(vfeat, efeat, v_reg_weight, v_reg_sum, e_reg_weight, e_reg_sum,
           node_idx, edge_idx, W1, b1, Wve, bve, Wev, bev,
           first_layer, last_layer):
    vfeat = np.asarray(vfeat, np.float32)
    NV, D = vfeat.shape
    NE = int(np.asarray(e_reg_weight).shape[0])
    node_idx = np.asarray(node_idx).astype(np.int64).ravel()
    edge_idx = np.asarray(edge_idx).astype(np.int64).ravel()

    # Host: weight fusion + per-message scalar weights (index metadata prep)
    W1f = np.asarray(W1, np.float32)
    Wvef = np.asarray(Wve, np.float32)
    Wc = (W1f @ Wvef)
    bcv = (np.asarray(b1, np.float32) @ Wvef + np.asarray(bve, np.float32))
    vrw = np.asarray(v_reg_weight, np.float32).ravel()
    vrs = np.asarray(v_reg_sum, np.float32).ravel()
    erw = np.asarray(e_reg_weight, np.float32).ravel()
    ers = np.asarray(e_reg_sum, np.float32).ravel()
    w_in = vrw[node_idx] / ers[edge_idx]
    w_con = erw[edge_idx] / vrs[node_idx]
    sumw = np.bincount(edge_idx, weights=w_in, minlength=NE).astype(np.float32)

    E_pc, N_pc = NE // NCORES, NV // NCORES
    WB = -(-E_pc // P)
    E_pad = WB * P
    WD = -(-N_pc // P)
    N_pad = WD * P

    oB = np.argsort(edge_idx, kind="stable")
    gidxB, elB, wB, TB, NT_B = _plan(
        edge_idx[oB], node_idx[oB].astype(np.int32), w_in[oB], E_pc, WB)
    oD = np.argsort(node_idx, kind="stable")
    ce = (edge_idx[oD] // E_pc).astype(np.int64)
    growD = (ce * E_pad + edge_idx[oD] - ce * E_pc).astype(np.int32)
    gidxD, elD, wD, TD, NT_D = _plan(
        node_idx[oD], growD, w_con[oD], N_pc, WD)

    key = (NV, NE, NT_B, NT_D, E_pad, N_pad, WB, TB, WD, TD)
    if key not in _CACHE:
        nc = _build(*key)
        _CACHE[key] = _Runner(nc)
    r = _CACHE[key]

    bf = ml_dtypes.bfloat16
    sumw_pad = np.zeros((NCORES, 1, E_pad), np.float32)
    sumw_pad[:, 0, :E_pc] = sumw.reshape(NCORES, E_pc)
    in_maps = []
    for c in range(NCORES):
        in_maps.append({
            "vfeat": vfeat,
            "gidxB": gidxB[c], "elB": elB[c], "wB": wB[c],
            "sumw": sumw_pad[c].astype(bf),
            "gidxD": gidxD[c], "elD": elD[c], "wD": wD[c],
            "Wc": Wc.astype(bf), "bc": bcv.reshape(1, P).astype(bf),
            "Wev": np.asarray(Wev, np.float32).astype(bf),
            "bev": np.asarray(bev, np.float32).reshape(1, P).astype(bf),
        })
    dev = r.put_inputs(in_maps)
    global _LAST_DEV
    _LAST_DEV = dev
    outs = r.run(dev)
    res = r.results(outs)

    feat_e = np.concatenate([res[c]["feT"].T[:E_pc] for c in range(NCORES)], 0)
    feat_v = np.concatenate([res[c]["fvout"][:N_pc] for c in range(NCORES)], 0)
    return feat_v.astype(np.float32), feat_e.astype(np.float32)
